# revision 1
# baseline (speedup 1.0000x reference)
"""Trainium2 Bass kernel for a Dirichlet-Process VI likelihood step.

Math (per reference):
  std  = log1p(exp(rho));  iv = 1/std^2
  quad[b,t]   = sum_d iv*(x-mu)^2 = sum_d iv*x^2 - 2*(mu*iv)*x + mu^2*iv
  kl_g[b,t]   = log_pdf + entropy = D/2 - 0.5*quad     (log-std terms cancel)
  log_pi[b,t] = log(beta) + exclusive-cumsum_t(log(1-beta))
  mix[t]      = N_pi / (N_g + N_pi),  N_* = sum over the FULL batch
  kl          = mix*kl_g + (1-mix)*log_pi
  out         = mean_b sum_t softmax_t(kl) * (mix*kl_g)

Distribution: data-parallel over batch (4096 -> 8 x 512 rows / core),
mu/rho replicated. One tiny [32,2] AllReduce carries the global N_g/N_pi
sums; the final per-core partial sums are combined on the host (the
unshard step).

On-chip layout is transposed ([T, batch]): every per-component broadcast
becomes a native per-partition scalar op, the stick-breaking cumsum is a
single triangular matmul, and the softmax reduction is a ones-matmul.
Softmax max-subtraction is skipped: kl is in [-74, -0.7] for this model,
exp() cannot overflow and the tiny terms underflow harmlessly.

"""

import os
import sys

import numpy as np

for _p in ("/opt/trn_rl_repo",):
    if os.path.isdir(_p) and _p not in sys.path:
        sys.path.insert(0, _p)

T = 32
D = 1024
B = 4096
NCORES = 8
BL = B // NCORES  # 512 batch rows per core
NJ = D // 128  # 8 contraction chunks of 128

# packed constants tensor layout: [128, 163]
#   cols 0:128   ident (128x128 identity)
#   cols 128:160 rows 0:64 = [Lstrict; I32] (cumsum + passthrough matmul)
#   cols 160:162 ones2 (rows 0:64) -- block-column selectors for den/num
#   col  162     ones128
CONSTW = 163


def _build_nc():
    import concourse.bacc as bacc
    import concourse.bass as bass
    import concourse.mybir as mybir
    import concourse.tile as tile

    f32 = mybir.dt.float32
    f32r = mybir.dt.float32r
    AF = mybir.ActivationFunctionType
    ALU = mybir.AluOpType

    nc = bacc.Bacc("TRN2", target_bir_lowering=False)

    x_d = nc.dram_tensor("x", [BL, D], f32, kind="ExternalInput").ap()
    beta_d = nc.dram_tensor("beta", [BL, T], f32, kind="ExternalInput").ap()
    mu_d = nc.dram_tensor("mu", [T, D], f32, kind="ExternalInput").ap()
    rho_d = nc.dram_tensor("rho", [T, D], f32, kind="ExternalInput").ap()
    consts_d = nc.dram_tensor("consts", [128, CONSTW], f32, kind="ExternalInput").ap()
    out_d = nc.dram_tensor("out", [1, 1], f32, kind="ExternalOutput").ap()

    with tile.TileContext(nc) as tc:
        with (
            tc.tile_pool(name="sb", bufs=1) as sb,
            tc.tile_pool(name="xpool", bufs=1) as xpool,
            tc.tile_pool(name="psx", bufs=4, space="PSUM") as psx,
            tc.tile_pool(name="psmisc", bufs=1, space="PSUM") as psmisc,
            tc.tile_pool(name="dram", bufs=1, space="DRAM") as dram,
        ):
            # ---------- input DMAs (independent -> sync HWDGE queue) ----------
            consts = sb.tile([128, CONSTW], f32, tag="consts")
            nc.sync.dma_start(consts[:], consts_d[:])
            ident = consts[:, 0:128]
            lcat = consts[0 : 2 * T, 128:160]
            ones2 = consts[0 : 2 * T, 160:162]
            ones128 = consts[:, 162:163]

            betab = sb.tile([128, 4, T], f32, tag="betab")
            nc.sync.dma_start(betab[:], beta_d.rearrange("(i p) t -> p i t", p=128))

            muf = sb.tile([128, 256], f32, tag="muf")
            nc.sync.dma_start(muf[:], mu_d.rearrange("t (s f) -> (t s) f", s=4))
            rhof = sb.tile([128, 256], f32, tag="rhof")
            nc.sync.dma_start(rhof[:], rho_d.rearrange("t (s f) -> (t s) f", s=4))

            xb = []
            for i in range(4):
                t_ = xpool.tile([128, D], f32, tag=f"xb{i}")
                nc.sync.dma_start(t_[:], x_d[128 * i : 128 * (i + 1), :])
                xb.append(t_)

            atl = mybir.InstLoadActFuncSet(
                name=nc.get_next_instruction_name(),
                ins=[],
                outs=[],
                act_func_set_id=6,
            )
            nc.scalar.add_instruction(atl)

            # ---------- beta path: betaT, log(beta), log(1-beta), cumsum ----------
            psB = psmisc.tile([T, BL], f32, tag="pss", bufs=2)
            for i in range(4):
                nc.tensor.transpose(
                    psB[:, 128 * i : 128 * (i + 1)], betab[:, i, :], ident
                )
            betaT = sb.tile([T, BL], f32, tag="betaT")
            nc.vector.tensor_copy(betaT[:], psB[:])
            # bcat = [ln(1-beta); ln(beta)] stacked on 64 partitions; the
            # [Lstrict; I32] stationary then yields log_pi^T in one matmul.
            bcat = sb.tile([2 * T, BL], f32, tag="bcat")
            nc.scalar.activation(bcat[0:T, :], betaT[:], AF.Ln, bias=1.0, scale=-1.0)
            nc.scalar.activation(bcat[T : 2 * T, :], betaT[:], AF.Ln)
            psC = psmisc.tile([T, BL], f32, tag="pss", bufs=2)
            nc.tensor.matmul(psC[:], lcat, bcat[:], start=True, stop=True)
            ccs = sb.tile([T, 2], f32, tag="ccs")
            lpiT = sb.tile([T, BL], f32, tag="lpiT")
            nc.vector.tensor_scalar(
                lpiT[:], psC[:], 0.0, 0.0, ALU.add, ALU.add,
                accum_out=ccs[:, 1:2],
            )

            # ---------- W prep on [128,256] folded layout ----------
            e1 = sb.tile([128, 256], f32, tag="e1")
            nc.scalar.activation(e1[:], rhof[:], AF.Exp)
            stdf = sb.tile([128, 256], f32, tag="stdf")
            nc.scalar.activation(stdf[:], e1[:], AF.Ln, bias=1.0)
            lstdf = sb.tile([128, 256], f32, tag="lstdf")
            nc.scalar.activation(lstdf[:], stdf[:], AF.Ln)
            ivf = sb.tile([128, 256], f32, tag="ivf")
            nc.scalar.activation(ivf[:], lstdf[:], AF.Exp, scale=-2.0)
            # w12f packs w1 (cols 0:256) and w2 (cols 256:512) for one bounce DMA
            w12f = sb.tile([128, 512], f32, tag="w12f")
            nc.vector.tensor_scalar(w12f[:, 0:256], ivf[:], -0.5, None, ALU.mult)
            nc.vector.tensor_tensor(w12f[:, 256:512], muf[:], ivf[:], ALU.mult)
            wtmp = sb.tile([128, 256], f32, tag="wtmp")
            m2r = sb.tile([128, 1], f32, tag="m2r")
            nc.vector.scalar_tensor_tensor(
                wtmp[:], muf[:], 1.0, w12f[:, 256:512], ALU.mult, ALU.mult,
                accum_out=m2r[:],
            )
            # unfold [128,256] -> [32,1024] via a DRAM bounce (row-major layouts match)
            wb12 = dram.tile([128, 512], f32, tag="wb12")
            nc.gpsimd.dma_start(wb12[:], w12f[:])
            mb = dram.tile([128, 1], f32, tag="mb")
            nc.gpsimd.dma_start(mb[:], m2r[:])
            wcat = sb.tile([T, 2 * D], f32, tag="wcat")
            nc.gpsimd.dma_start(
                wcat[:, 0:D].rearrange("t (s f) -> t s f", s=4),
                wb12[:, 0:256].rearrange("(t s) f -> t s f", s=4),
            )
            nc.gpsimd.dma_start(
                wcat[:, D : 2 * D].rearrange("t (s f) -> t s f", s=4),
                wb12[:, 256:512].rearrange("(t s) f -> t s f", s=4),
            )
            c4 = sb.tile([T, 4], f32, tag="c4")
            nc.gpsimd.dma_start(c4[:], mb[:].rearrange("(t s) o -> t (s o)", s=4))
            c1 = sb.tile([T, 1], f32, tag="c1")
            nc.vector.reduce_sum(c1[:], c4[:], axis=mybir.AxisListType.X)
            k0 = sb.tile([T, 1], f32, tag="k0")
            nc.vector.tensor_scalar(k0[:], c1[:], -0.5, float(D // 2), ALU.mult, ALU.add)

            # WT[:, 32k:32k+32] = (wcat[:, 128k:128k+128]).T ; k<8 -> W1T, k>=8 -> W2T
            psW = psmisc.tile([128, 16 * T], f32, tag="psw", bufs=1)
            for k in range(16):
                nc.tensor.transpose(
                    psW[:, T * k : T * (k + 1)],
                    wcat[:, 128 * k : 128 * (k + 1)],
                    ident[0:T, 0:T],
                )
            WT = sb.tile([128, 16 * T], f32r, tag="WT")
            nc.vector.tensor_copy(WT[:], psW[:])

            # ---------- x transposes, PSUM drains, squares ----------
            # Drain copies & squares also accumulate per-tile batch colsums
            # (cs1 = sum_b x^T, cs2 = sum_b (x^2)^T) so N_g can be computed
            # before the GEMM finishes and the AllReduce overlaps it.
            xT = {}
            xxT = {}
            sq_cycle = 0
            for h in range(2):
                for j in range(NJ):
                    c = 2 * j + h
                    pst = psx.tile([128, 256], f32, tag="pst", padded_shape=[128, 512])
                    nc.tensor.transpose(
                        pst[:, 0:128],
                        xb[2 * h][:, 128 * j : 128 * (j + 1)],
                        ident,
                    )
                    nc.tensor.transpose(
                        pst[:, 128:256],
                        xb[2 * h + 1][:, 128 * j : 128 * (j + 1)],
                        ident,
                    )
                    xt = xpool.tile([128, 256], f32r, tag=f"xT{j}_{h}")
                    xx = xpool.tile([128, 256], f32r, tag=f"xxT{j}_{h}")
                    if (j + h) % 4 != 3:
                        nc.vector.tensor_copy(xt[:], pst[:])
                    else:
                        nc.scalar.copy(xt[:], pst[:])
                    sq = sq_cycle % 4
                    sq_cycle += 1
                    if sq in (0, 2):
                        nc.gpsimd.tensor_tensor(xx[:], xt[:], xt[:], ALU.mult)
                    elif sq == 1:
                        nc.scalar.square(xx[:], xt[:])
                    else:
                        nc.vector.tensor_tensor(xx[:], xt[:], xt[:], ALU.mult)
                    xT[(j, h)] = xt
                    xxT[(j, h)] = xx

            # ---------- main GEMM: psG[t, b] = sum_d W1T*xx + W2T*x ----------
            psG = psmisc.tile([T, BL], f32, tag="psg", bufs=1)
            for h in range(2):
                g = psG[:, 256 * h : 256 * (h + 1)]
                for j in range(NJ):
                    nc.tensor.matmul(
                        g,
                        WT[:, T * j : T * (j + 1)],
                        xxT[(j, h)][:],
                        start=(j == 0),
                        stop=False,
                    )
                    nc.tensor.matmul(
                        g,
                        WT[:, T * (NJ + j) : T * (NJ + j + 1)],
                        xT[(j, h)][:],
                        start=False,
                        stop=(j == NJ - 1),
                    )

            # ---------- kl_g^T (+k0) and its batch-sum ----------
            klgT = sb.tile([T, BL], f32, tag="klgT")
            nc.vector.tensor_scalar(
                klgT[:], psG[:], k0[:], 0.0, ALU.add, ALU.add,
                accum_out=ccs[:, 0:1],
            )

            # ---------- AllReduce of [T,2] partial sums ----------
            ccin = dram.tile([T, 2], f32, tag="ccin")
            ccout = dram.tile([T, 2], f32, tag="ccout")
            nc.gpsimd.dma_start(ccin[:], ccs[:])
            if os.environ.get("DPVI_NO_CC"):
                nc.gpsimd.dma_start(ccout[:], ccin[:])
            else:
                nc.gpsimd.collective_compute(
                    "AllReduce",
                    ALU.add,
                    replica_groups=[list(range(NCORES))],
                    ins=[ccin.opt()],
                    outs=[ccout.opt()],
                )
            red = sb.tile([T, 2], f32, tag="red")
            nc.gpsimd.dma_start(red[:], ccout[:])

            ssum = sb.tile([T, 1], f32, tag="ssum")
            nc.vector.tensor_tensor(ssum[:], red[:, 0:1], red[:, 1:2], ALU.add)
            rinv = sb.tile([T, 1], f32, tag="rinv")
            nc.vector.reciprocal(rinv[:], ssum[:])
            mix = sb.tile([T, 1], f32, tag="mix")
            nc.vector.tensor_tensor(mix[:], red[:, 1:2], rinv[:], ALU.mult)
            omix = sb.tile([T, 1], f32, tag="omix")
            nc.vector.tensor_tensor(omix[:], red[:, 0:1], rinv[:], ALU.mult)

            # ---------- kl, exp, weighted sums ----------
            bb = sb.tile([T, BL], f32, tag="bb")
            nc.vector.tensor_scalar(bb[:], lpiT[:], omix[:], None, ALU.mult)
            kl = sb.tile([T, BL], f32, tag="kl")
            nc.vector.scalar_tensor_tensor(
                kl[:], klgT[:], mix[:], bb[:], ALU.mult, ALU.add
            )
            s64 = sb.tile([2 * T, BL], f32, tag="s64")
            nc.scalar.activation(s64[0:T, :], kl[:], AF.Exp)
            nc.vector.scalar_tensor_tensor(
                s64[T : 2 * T, :], klgT[:], mix[:], s64[0:T, :], ALU.mult, ALU.mult
            )
            # psD[0,:] = sum_t exp(kl) (den), psD[1,:] = sum_t exp(kl)*mix*kl_g (num)
            psD = psmisc.tile([2, BL], f32, tag="pss", bufs=2)
            nc.tensor.matmul(psD[:], ones2, s64[:], start=True, stop=True)
            nd = sb.tile([2, BL], f32, tag="nd")
            nc.vector.tensor_copy(nd[:], psD[:])

            # transpose den/num back to [128, 2c] so the division uses 128 lanes
            psTr = psmisc.tile([128, 8], f32, tag="pss", bufs=2)
            for c in range(4):
                nc.tensor.transpose(
                    psTr[:, 2 * c : 2 * c + 2],
                    nd[:, 128 * c : 128 * (c + 1)],
                    ident[0:2, 0:2],
                )
            rd = sb.tile([128, 4], f32, tag="rd")
            nc.vector.reciprocal(rd[:], psTr[:, 0:8:2])
            liks = sb.tile([128, 1], f32, tag="liks")
            likv = sb.tile([128, 4], f32, tag="likv")
            nc.vector.scalar_tensor_tensor(
                likv[:], psTr[:, 1:8:2], 1.0, rd[:], ALU.mult, ALU.mult,
                accum_out=liks[:],
            )
            psL = psmisc.tile([1, 1], f32, tag="pss", bufs=2)
            nc.tensor.matmul(psL[:], ones128, liks[:], start=True, stop=True)
            outsb = sb.tile([1, 1], f32, tag="outsb")
            nc.vector.tensor_copy(outsb[:], psL[:])
            nc.gpsimd.dma_start(out_d[:], outsb[:])

    nc.compile()
    return nc


_NC_CACHE = None


def _get_nc():
    global _NC_CACHE
    if _NC_CACHE is None:
        _NC_CACHE = _build_nc()
    return _NC_CACHE


def _make_in_maps(x, mu, rho, beta_samples):
    x = np.ascontiguousarray(x, dtype=np.float32)
    mu = np.ascontiguousarray(mu, dtype=np.float32)
    rho = np.ascontiguousarray(rho, dtype=np.float32)
    beta = np.ascontiguousarray(beta_samples, dtype=np.float32)

    consts = np.zeros((128, CONSTW), dtype=np.float32)
    consts[:, 0:128] = np.eye(128, dtype=np.float32)
    consts[0:T, 128:160] = np.triu(np.ones((T, T), np.float32), 1)  # k<m
    consts[T : 2 * T, 128:160] = np.eye(T, dtype=np.float32)
    consts[0:T, 160] = 1.0  # den selector
    consts[T : 2 * T, 161] = 1.0  # num selector
    consts[:, 162] = 1.0  # ones128

    in_maps = []
    for c in range(NCORES):
        in_maps.append(
            {
                "x": x[BL * c : BL * (c + 1)],
                "beta": beta[BL * c : BL * (c + 1)],
                "mu": mu,
                "rho": rho,
                "consts": consts,
            }
        )
    return in_maps


def run(inputs, trace=False, **kw):
    """Run on 8 NeuronCores; returns (result_scalar, BassKernelResults)."""
    from concourse.bass_utils import run_bass_kernel_spmd

    nc = _get_nc()
    in_maps = _make_in_maps(**inputs)
    res = run_bass_kernel_spmd(
        nc, in_maps, core_ids=list(range(NCORES)), trace=trace, **kw
    )
    total = 0.0
    for c in range(NCORES):
        total += float(res.results[c]["out"][0, 0])
    value = np.float32(total / B).reshape(())
    return value, res


def kernel(x, mu, rho, beta_samples):
    value, _ = run(dict(x=x, mu=mu, rho=rho, beta_samples=beta_samples))
    return value



# revision 12
# speedup vs baseline: 1.0960x; 1.0960x over previous
"""Trainium2 Bass kernel for a Dirichlet-Process VI likelihood step.

Math (per reference):
  std  = log1p(exp(rho));  iv = 1/std^2
  quad[b,t]   = sum_d iv*x^2 - 2*(mu*iv)*x + mu^2*iv
  kl_g[b,t]   = log_pdf + entropy = D/2 - 0.5*quad     (log-std terms cancel)
  log_pi[b,t] = log(beta) + exclusive-cumsum_t(log(1-beta))
  mix[t]      = N_pi / (N_g + N_pi),  N_* = sum over the FULL batch
  kl          = mix*kl_g + (1-mix)*log_pi = log_pi + mix*(kl_g - log_pi)
  out         = mean_b sum_t softmax_t(kl) * (mix*kl_g)

Distribution: data-parallel over batch (4096 -> 8 x 512 rows / core),
mu/rho replicated. A tiny [32,2] AllReduce carries the global N_g/N_pi
sums; per-core scalar partials are combined on the host (unshard).

v2 structure (vs v1):
  - W prep unfolds on-chip (TensorE transposes + strided drains) instead
    of bouncing through DRAM (v1 stalled ~12us on that bounce).
  - The main GEMM is interleaved with the x transposes on TensorE, so it
    finishes right after the last x tile instead of serially after.
  - N_g / N_pi partials fall out of the PSUM-drain ops' accum_out, so
    the AllReduce triggers ~25us earlier than v1; the stick-breaking
    epilogue and softmax prep execute inside the collective's latency.
  - The whole tail runs on 128 partitions ([4*T, BL/4] stacking).
"""

import os
import sys

import numpy as np

for _p in ("/opt/trn_rl_repo",):
    if os.path.isdir(_p) and _p not in sys.path:
        sys.path.insert(0, _p)

T = 32
D = 1024
B = 4096
NCORES = 8
BL = B // NCORES  # 512 batch rows per core
NJ = D // 128  # 8 contraction chunks of 128

# packed constants tensor layout: [128, 196]
#   cols 0:128   ident (128x128 identity, f32)
#   cols 128:160 rows 0:64 = [Lstrict; I32] (cumsum matmul stationary)
#   cols 160:192 sel4: sel4[p, p//4] = 1   (fold m2 over s -> [T])
#   cols 192:196 selblk: selblk[p, p//32] = 1  (tail den/num reduction;
#                 col 0 doubles as ones4 for the final sum)
CONSTW = 196


def _build_nc():
    import concourse.bacc as bacc
    import concourse.bass as bass
    import concourse.mybir as mybir
    import concourse.tile as tile

    f32 = mybir.dt.float32
    f32r = mybir.dt.float32r
    AF = mybir.ActivationFunctionType
    ALU = mybir.AluOpType

    nc = bacc.Bacc("TRN2", target_bir_lowering=False)

    x_d = nc.dram_tensor("x", [BL, D], f32, kind="ExternalInput").ap()
    beta_d = nc.dram_tensor("beta", [BL, T], f32, kind="ExternalInput").ap()
    mu_d = nc.dram_tensor("mu", [T, D], f32, kind="ExternalInput").ap()
    rho_d = nc.dram_tensor("rho", [T, D], f32, kind="ExternalInput").ap()
    consts_d = nc.dram_tensor("consts", [128, CONSTW], f32, kind="ExternalInput").ap()
    out_d = nc.dram_tensor("out", [1, 1], f32, kind="ExternalOutput").ap()

    with tile.TileContext(nc) as tc:
        with (
            tc.tile_pool(name="sb", bufs=1) as sb,
            tc.tile_pool(name="xpool", bufs=1) as xpool,
            tc.tile_pool(name="psx", bufs=4, space="PSUM") as psx,
            tc.tile_pool(name="psmisc", bufs=1, space="PSUM") as psmisc,
            tc.tile_pool(name="dram", bufs=1, space="DRAM") as dram,
        ):
            # ---------- input DMAs ----------
            # consts first (transposes need ident), then x serially on the
            # sync queue so early tiles land early; small inputs on gpsimd.
            consts = sb.tile([128, CONSTW], f32, tag="consts")
            nc.sync.dma_start(consts[:], consts_d[:])
            ident = consts[:, 0:128]
            sel4 = consts[:, 160:192]
            ones4 = consts[0:4, 192:193]

            xb = []
            for i in range(4):
                t_ = xpool.tile([128, D], f32, tag=f"xb{i}")
                nc.sync.dma_start(t_[:], x_d[128 * i : 128 * (i + 1), :])
                xb.append(t_)

            # f32r view of the small stationaries (same bits, single-pass mm)
            selr = sb.tile([128, 68], f32r, tag="selr")
            nc.gpsimd.dma_start(selr[:], consts_d[:, 128:196])
            lcatr = selr[0 : 2 * T, 0:32]
            selblkr = selr[:, 64:68]

            betab = sb.tile([128, 4, T], f32, tag="betab")
            nc.gpsimd.dma_start(betab[:], beta_d.rearrange("(i p) t -> p i t", p=128))
            rhof = sb.tile([128, 256], f32, tag="rhof")
            nc.gpsimd.dma_start(rhof[:], rho_d.rearrange("t (s f) -> (t s) f", s=4))
            muf = sb.tile([128, 256], f32, tag="muf")
            nc.gpsimd.dma_start(muf[:], mu_d.rearrange("t (s f) -> (t s) f", s=4))

            atl = mybir.InstLoadActFuncSet(
                name=nc.get_next_instruction_name(),
                ins=[],
                outs=[],
                act_func_set_id=6,
            )
            nc.scalar.add_instruction(atl)

            # ---------- W prep scalar chain (first in the scalar queue) ----
            e1 = sb.tile([128, 256], f32, tag="e1")
            nc.scalar.activation(e1[:], rhof[:], AF.Exp)
            stdf = sb.tile([128, 256], f32, tag="stdf")
            nc.scalar.activation(stdf[:], e1[:], AF.Ln, bias=1.0)
            lstdf = sb.tile([128, 256], f32, tag="lstdf")
            nc.scalar.activation(lstdf[:], stdf[:], AF.Ln)
            ivf = sb.tile([128, 256], f32, tag="ivf")
            nc.scalar.activation(ivf[:], lstdf[:], AF.Exp, scale=-2.0)

            # ---------- beta transposes ----------
            psB = psmisc.tile([T, BL], f32, tag="pss", bufs=2)
            for i in range(4):
                nc.tensor.transpose(
                    psB[:, 128 * i : 128 * (i + 1)], betab[:, i, :], ident
                )
            betaT = sb.tile([T, BL], f32, tag="betaT")
            nc.vector.tensor_copy(betaT[:], psB[:])
            # bcat = [ln(1-beta); ln(beta)] on 64 partitions
            bcat = sb.tile([2 * T, BL], f32r, tag="bcat")
            nc.scalar.activation(bcat[0:T, :], betaT[:], AF.Ln, bias=1.0, scale=-1.0)
            nc.scalar.activation(bcat[T : 2 * T, :], betaT[:], AF.Ln)

            # ---------- x transposes + drains + interleaved main GEMM ------
            xT = {}
            xxT = {}
            pst_tiles = {}
            psG = psmisc.tile([T, BL], f32, tag="psg", bufs=1)

            def emit_transpose(j, h):
                pst = psx.tile([128, 256], f32, tag="pst", padded_shape=[128, 512])
                nc.tensor.transpose(
                    pst[:, 0:128], xb[2 * h][:, 128 * j : 128 * (j + 1)], ident
                )
                nc.tensor.transpose(
                    pst[:, 128:256], xb[2 * h + 1][:, 128 * j : 128 * (j + 1)], ident
                )
                pst_tiles[(j, h)] = pst

            def emit_drains(j, h):
                pst = pst_tiles[(j, h)]
                xt = xpool.tile([128, 256], f32r, tag=f"xT{j}_{h}")
                xx = xpool.tile([128, 256], f32r, tag=f"xxT{j}_{h}")
                # each drain reads PSUM once (two-PSUM-input ops are illegal,
                # and gpsimd cannot read PSUM at all)
                nc.vector.tensor_copy(xt[:], pst[:])
                nc.scalar.activation(xx[:], pst[:], AF.Square)
                xT[(j, h)] = xt
                xxT[(j, h)] = xx

            def emit_gemm(j, h):
                g = psG[:, 256 * h : 256 * (h + 1)]
                nc.tensor.matmul(
                    g, WT[:, 64 * j + 32 : 64 * j + 64], xxT[(j, h)][:],
                    start=(j == 0), stop=False,
                )
                nc.tensor.matmul(
                    g, WT[:, 64 * j : 64 * j + 32], xT[(j, h)][:],
                    start=False, stop=(j == NJ - 1),
                )

            WT = sb.tile([128, 512], f32r, tag="WT")

            order = [(j, h) for h in range(2) for j in range(NJ)]
            for (j, h) in order[:2]:
                emit_transpose(j, h)
                emit_drains(j, h)

            # W vector part, emitted after the first drains so the early
            # drains don't queue behind it on the vector engine.
            # w12f: cols 0:256 = w1 = -0.5*iv, cols 256:512 = w2 = mu*iv
            w12f = sb.tile([128, 512], f32, tag="w12f")
            nc.vector.tensor_scalar(w12f[:, 0:256], ivf[:], -0.5, None, ALU.mult)
            nc.vector.tensor_tensor(w12f[:, 256:512], muf[:], ivf[:], ALU.mult)
            wtmp = sb.tile([128, 256], f32, tag="wtmp")
            m2r = sb.tile([128, 1], f32, tag="m2r")
            nc.vector.scalar_tensor_tensor(
                wtmp[:], muf[:], 1.0, w12f[:, 256:512], ALU.mult, ALU.mult,
                accum_out=m2r[:],
            )

            for (j, h) in order[2:4]:
                emit_transpose(j, h)
                emit_drains(j, h)
            # cumsum matmul here: bcat is ready by now, TensorE doesn't stall
            psC = psmisc.tile([T, BL], f32, tag="pss", bufs=2)
            nc.tensor.matmul(psC[:], lcatr, bcat[:], start=True, stop=True)
            for (j, h) in order[4:6]:
                emit_transpose(j, h)
                emit_drains(j, h)

            # ---------- W unfold via on-chip transposes ----------
            # psWF chunk c' (128 cols) = transpose of w12f[:, 128c':128c'+128];
            # within a chunk, column index = (t*4 + s).
            psWF = psmisc.tile([128, 512], f32, tag="psw", bufs=1)
            for cpr in range(4):
                nc.tensor.transpose(
                    psWF[:, 128 * cpr : 128 * (cpr + 1)],
                    w12f[:, 128 * cpr : 128 * (cpr + 1)],
                    ident,
                )
            # k0 fold: psK = sum_s m2r[(t s)]  (plain f32: 1-col f32r mm is
            # rejected by the ISA checker)
            psK = psmisc.tile([T, 1], f32, tag="pss", bufs=2)
            nc.tensor.matmul(psK[:], sel4, m2r[:], start=True, stop=True)
            k0 = sb.tile([T, 1], f32, tag="k0")
            nc.vector.tensor_scalar(k0[:], psK[:], -0.5, float(D // 2), ALU.mult, ALU.add)

            # WT col layout per d-chunk j (= 2s + fh):
            #   [64j : 64j+32]    = w2T_j
            #   [64j+32 : 64j+64] = w1T_j
            wtv = WT.rearrange("p (s x) -> p s x", x=128)
            for kindbit in range(2):  # 0 = w2, 1 = w1
                for fh in range(2):
                    cpr = 2 * (1 - kindbit) + fh
                    src = psWF[:, 128 * cpr : 128 * (cpr + 1)].rearrange(
                        "p (t s) -> p s t", s=4
                    )
                    off = 64 * fh + 32 * kindbit
                    nc.vector.tensor_copy(wtv[:, :, off : off + 32], src)

            # rest of the transposes, GEMM chasing with lag 4
            LAG = 4
            gcur = 0
            for i, (j, h) in enumerate(order[6:], start=6):
                emit_transpose(j, h)
                emit_drains(j, h)
                while gcur <= i - LAG:
                    emit_gemm(*order[gcur])
                    gcur += 1
            while gcur < len(order):
                emit_gemm(*order[gcur])
                gcur += 1

            # ---------- lpi4 (stick-breaking, 128-partition stacked) ----------
            lpi4 = sb.tile([128, BL // 4], f32, tag="lpi4")
            npacc = sb.tile([T, 4], f32, tag="npacc")
            for blk in range(4):
                nc.vector.tensor_scalar(
                    lpi4[T * blk : T * (blk + 1), :],
                    psC[:, 128 * blk : 128 * (blk + 1)],
                    0.0, 0.0, ALU.add, ALU.add,
                    accum_out=npacc[:, blk : blk + 1],
                )

            # klg4 stacked [128, 128] w/ batch sums for N_g
            klg4 = sb.tile([128, BL // 4], f32, tag="klg4")
            ngacc = sb.tile([T, 4], f32, tag="ngacc")
            for blk in range(4):
                nc.vector.tensor_scalar(
                    klg4[T * blk : T * (blk + 1), :],
                    psG[:, 128 * blk : 128 * (blk + 1)],
                    k0[:], 0.0, ALU.add, ALU.add,
                    accum_out=ngacc[:, blk : blk + 1],
                )

            # ---------- assemble [T,2] partials, AllReduce ----------
            ccs = sb.tile([T, 2], f32, tag="ccs")
            nc.vector.reduce_sum(ccs[:, 0:1], ngacc[:], axis=mybir.AxisListType.X)
            nc.vector.reduce_sum(ccs[:, 1:2], npacc[:], axis=mybir.AxisListType.X)

            ccin = dram.tile([T, 2], f32, tag="ccin")
            ccout = dram.tile([T, 2], f32, tag="ccout")
            nc.gpsimd.dma_start(ccin[:], ccs[:])
            if os.environ.get("DPVI_NO_CC"):
                nc.gpsimd.dma_start(ccout[:], ccin[:])
            else:
                nc.gpsimd.collective_compute(
                    "AllReduce",
                    ALU.add,
                    replica_groups=[list(range(NCORES))],
                    ins=[ccin.opt()],
                    outs=[ccout.opt()],
                )

            # diff4 overlaps the collective
            diff4 = sb.tile([128, BL // 4], f32, tag="diff4")
            nc.vector.tensor_tensor(diff4[:], klg4[:], lpi4[:], ALU.subtract)

            # ---------- post-collective tail ----------
            red4 = sb.tile([128, 2], f32, tag="red4")
            nc.gpsimd.dma_start(red4[0:T, :], ccout[:])
            nc.vector.tensor_copy(red4[T : 2 * T, :], red4[0:T, :])
            nc.vector.tensor_copy(red4[2 * T : 4 * T, :], red4[0 : 2 * T, :])

            rsum = sb.tile([128, 1], f32, tag="rsum")
            nc.vector.tensor_tensor(rsum[:], red4[:, 0:1], red4[:, 1:2], ALU.add)
            rinv = sb.tile([128, 1], f32, tag="rinv")
            nc.vector.reciprocal(rinv[:], rsum[:])
            mix4 = sb.tile([128, 1], f32, tag="mix4")
            nc.vector.tensor_tensor(mix4[:], red4[:, 1:2], rinv[:], ALU.mult)

            # kl = log_pi + mix*(kl_g - log_pi); phi-num = exp(kl)
            kl4 = sb.tile([128, BL // 4], f32, tag="kl4")
            nc.vector.scalar_tensor_tensor(
                kl4[:], diff4[:], mix4[:], lpi4[:], ALU.mult, ALU.add
            )
            e4 = sb.tile([128, BL // 4], f32r, tag="e4")
            nc.scalar.activation(e4[:], kl4[:], AF.Exp)
            mkl4 = sb.tile([128, BL // 4], f32r, tag="mkl4")
            nc.vector.scalar_tensor_tensor(
                mkl4[:], klg4[:], mix4[:], e4[:], ALU.mult, ALU.mult
            )

            # den / num per batch column (separate PSUM tiles: matmul out
            # base partition must be 0/32/64)
            psDe = psmisc.tile([4, BL // 4], f32, tag="pss", bufs=2)
            nc.tensor.matmul(psDe[:], selblkr, e4[:], start=True, stop=True)
            psDm = psmisc.tile([4, BL // 4], f32, tag="pss", bufs=2)
            nc.tensor.matmul(psDm[:], selblkr, mkl4[:], start=True, stop=True)
            nde = sb.tile([4, BL // 4], f32, tag="nde")
            nc.vector.tensor_copy(nde[:], psDe[:])
            rd = sb.tile([4, BL // 4], f32, tag="rd")
            nc.vector.reciprocal(rd[:], nde[:])
            ndn = sb.tile([4, BL // 4], f32, tag="ndn")
            nc.vector.tensor_copy(ndn[:], psDm[:])
            lik4 = sb.tile([4, BL // 4], f32, tag="lik4")
            liks = sb.tile([4, 1], f32, tag="liks")
            nc.vector.scalar_tensor_tensor(
                lik4[:], ndn[:], 1.0, rd[:], ALU.mult, ALU.mult,
                accum_out=liks[:],
            )
            psL = psmisc.tile([1, 1], f32, tag="pss", bufs=2)
            nc.tensor.matmul(psL[:], liks[:], ones4, start=True, stop=True)
            outsb = sb.tile([1, 1], f32, tag="outsb")
            nc.vector.tensor_copy(outsb[:], psL[:])
            nc.gpsimd.dma_start(out_d[:], outsb[:])

    nc.compile()
    return nc


_NC_CACHE = None


def _get_nc():
    global _NC_CACHE
    if _NC_CACHE is None:
        _NC_CACHE = _build_nc()
    return _NC_CACHE


def _make_in_maps(x, mu, rho, beta_samples):
    x = np.ascontiguousarray(x, dtype=np.float32)
    mu = np.ascontiguousarray(mu, dtype=np.float32)
    rho = np.ascontiguousarray(rho, dtype=np.float32)
    beta = np.ascontiguousarray(beta_samples, dtype=np.float32)

    consts = np.zeros((128, CONSTW), dtype=np.float32)
    consts[:, 0:128] = np.eye(128, dtype=np.float32)
    consts[0:T, 128:160] = np.triu(np.ones((T, T), np.float32), 1)  # k<t
    consts[T : 2 * T, 128:160] = np.eye(T, dtype=np.float32)
    p = np.arange(128)
    consts[p, 160 + p // 4] = 1.0  # sel4
    consts[p, 192 + p // 32] = 1.0  # selblk / ones4

    in_maps = []
    for c in range(NCORES):
        in_maps.append(
            {
                "x": x[BL * c : BL * (c + 1)],
                "beta": beta[BL * c : BL * (c + 1)],
                "mu": mu,
                "rho": rho,
                "consts": consts,
            }
        )
    return in_maps


def run(inputs, trace=False, **kw):
    """Run on 8 NeuronCores; returns (result_scalar, BassKernelResults)."""
    from concourse.bass_utils import run_bass_kernel_spmd

    nc = _get_nc()
    in_maps = _make_in_maps(**inputs)
    res = run_bass_kernel_spmd(
        nc, in_maps, core_ids=list(range(NCORES)), trace=trace, **kw
    )
    total = 0.0
    for c in range(NCORES):
        total += float(res.results[c]["out"][0, 0])
    value = np.float32(total / B).reshape(())
    return value, res


def kernel(x, mu, rho, beta_samples):
    value, _ = run(dict(x=x, mu=mu, rho=rho, beta_samples=beta_samples))
    return value


# revision 17
# speedup vs baseline: 1.1887x; 1.0845x over previous
"""Trainium2 Bass kernel for a Dirichlet-Process VI likelihood step.

Math (per reference):
  std  = log1p(exp(rho));  iv = 1/std^2
  quad[b,t]   = sum_d iv*x^2 - 2*(mu*iv)*x + mu^2*iv
  kl_g[b,t]   = log_pdf + entropy = D/2 - 0.5*quad     (log-std terms cancel)
  log_pi[b,t] = log(beta) + exclusive-cumsum_t(log(1-beta))
  mix[t]      = N_pi / (N_g + N_pi),  N_* = sum over the FULL batch
  kl          = mix*kl_g + (1-mix)*log_pi = log_pi + mix*(kl_g - log_pi)
  out         = mean_b sum_t softmax_t(kl) * (mix*kl_g)

Distribution: data-parallel over batch (4096 -> 8 x 512 rows / core),
mu/rho replicated. A tiny [32,2] AllReduce carries the global N_g/N_pi
sums; per-core scalar partials are combined on the host (unshard).

v2 structure (vs v1):
  - W prep unfolds on-chip (TensorE transposes + strided drains) instead
    of bouncing through DRAM (v1 stalled ~12us on that bounce).
  - The main GEMM is interleaved with the x transposes on TensorE, so it
    finishes right after the last x tile instead of serially after.
  - N_g / N_pi partials fall out of the PSUM-drain ops' accum_out, so
    the AllReduce triggers ~25us earlier than v1; the stick-breaking
    epilogue and softmax prep execute inside the collective's latency.
  - The whole tail runs on 128 partitions ([4*T, BL/4] stacking).
"""

import os
import sys

import numpy as np

for _p in ("/opt/trn_rl_repo",):
    if os.path.isdir(_p) and _p not in sys.path:
        sys.path.insert(0, _p)

T = 32
D = 1024
B = 4096
NCORES = 8
BL = B // NCORES  # 512 batch rows per core
NJ = D // 128  # 8 contraction chunks of 128

# packed constants tensor layout: [128, 196]
#   cols 0:128   ident (128x128 identity, f32)
#   cols 128:160 rows 0:64 = [Lstrict; I32] (cumsum matmul stationary)
#   cols 160:192 sel4: sel4[p, p//4] = 1   (fold m2 over s -> [T])
#   cols 192:196 selblk: selblk[p, p//32] = 1  (tail den/num reduction;
#                 col 0 doubles as ones4 for the final sum)
CONSTW = 196


def _build_nc():
    import concourse.bacc as bacc
    import concourse.bass as bass
    import concourse.mybir as mybir
    import concourse.tile as tile

    f32 = mybir.dt.float32
    f32r = mybir.dt.float32r
    AF = mybir.ActivationFunctionType
    ALU = mybir.AluOpType

    nc = bacc.Bacc("TRN2", target_bir_lowering=False)

    x_d = nc.dram_tensor("x", [BL, D], f32, kind="ExternalInput").ap()
    beta_d = nc.dram_tensor("beta", [BL, T], f32, kind="ExternalInput").ap()
    mu_d = nc.dram_tensor("mu", [T, D], f32, kind="ExternalInput").ap()
    rho_d = nc.dram_tensor("rho", [T, D], f32, kind="ExternalInput").ap()
    consts_d = nc.dram_tensor("consts", [128, CONSTW], f32, kind="ExternalInput").ap()
    out_d = nc.dram_tensor("out", [1, 1], f32, kind="ExternalOutput").ap()

    with tile.TileContext(nc) as tc:
        with (
            tc.tile_pool(name="sb", bufs=1) as sb,
            tc.tile_pool(name="xpool", bufs=1) as xpool,
            tc.tile_pool(name="psx", bufs=4, space="PSUM") as psx,
            tc.tile_pool(name="psmisc", bufs=1, space="PSUM") as psmisc,
            tc.tile_pool(name="dram", bufs=1, space="DRAM") as dram,
        ):
            # ---------- input DMAs ----------
            # consts first (transposes need ident), then x serially on the
            # sync queue so early tiles land early; small inputs on gpsimd.
            consts = sb.tile([128, CONSTW], f32, tag="consts")
            nc.sync.dma_start(consts[:], consts_d[:])
            ident = consts[:, 0:128]
            sel4 = consts[:, 160:192]
            ones4 = consts[0:4, 192:193]

            xb = []
            for i in range(4):
                t_ = xpool.tile([128, D], f32, tag=f"xb{i}")
                nc.sync.dma_start(t_[:], x_d[128 * i : 128 * (i + 1), :])
                xb.append(t_)

            # f32r view of the small stationaries (same bits, single-pass mm)
            selr = sb.tile([128, 68], f32r, tag="selr")
            nc.gpsimd.dma_start(selr[:], consts_d[:, 128:196])
            lcatr = selr[0 : 2 * T, 0:32]
            selblkr = selr[:, 64:68]

            betab = sb.tile([128, 4, T], f32, tag="betab")
            nc.gpsimd.dma_start(betab[:], beta_d.rearrange("(i p) t -> p i t", p=128))
            rhof = sb.tile([128, 256], f32, tag="rhof")
            nc.gpsimd.dma_start(rhof[:], rho_d.rearrange("t (s f) -> (t s) f", s=4))
            muf = sb.tile([128, 256], f32, tag="muf")
            nc.gpsimd.dma_start(muf[:], mu_d.rearrange("t (s f) -> (t s) f", s=4))

            atl = mybir.InstLoadActFuncSet(
                name=nc.get_next_instruction_name(),
                ins=[],
                outs=[],
                act_func_set_id=6,
            )
            nc.scalar.add_instruction(atl)

            # ---------- remote-DMA allgather prep (v3 path) ----------
            # Each core broadcasts its replicated [128,2] partial straight
            # into slot d of every peer's gather buffer via SWDGE remote DMA
            # (peer for slot d = own_tpb XOR d), skipping the collective
            # firmware entirely: no entry barrier, no ncfw dispatch latency.
            # Descriptors are generated here (expensive, ~1us each on the
            # Q7) and fired later with trigger_dma once ccs128 is ready.
            # Default: firmware collective. DPVI_REMOTE=1 selects the direct
            # remote-DMA allgather — correct by construction, but the Tile
            # scheduling simulator cannot model remotely-incremented
            # semaphores and rejects the graph as a deadlock, so it stays
            # experimental.
            use_remote = bool(os.environ.get("DPVI_REMOTE"))
            if use_remote:
                rsem = nc.alloc_semaphore("dpvi_rsem")
                lsem = nc.alloc_semaphore("dpvi_lsem")
                ccs128 = sb.tile([128, 2], f32, tag="ccs128")
                gbuf = sb.tile([128, 16], f32, tag="gbuf")
                for dpeer in range(1, 8):
                    rdests = [None] * 8
                    rdests[dpeer] = (0, dpeer)
                    nc.gpsimd.remote_dma_broadcast(
                        gbuf[:, 2 * dpeer : 2 * dpeer + 2],
                        ccs128[:],
                        rsem,
                        lsem,
                        rdests=rdests,
                    )

            # ---------- W prep scalar chain (first in the scalar queue) ----
            e1 = sb.tile([128, 256], f32, tag="e1")
            nc.scalar.activation(e1[:], rhof[:], AF.Exp)
            stdf = sb.tile([128, 256], f32, tag="stdf")
            nc.scalar.activation(stdf[:], e1[:], AF.Ln, bias=1.0)
            lstdf = sb.tile([128, 256], f32, tag="lstdf")
            nc.scalar.activation(lstdf[:], stdf[:], AF.Ln)
            ivf = sb.tile([128, 256], f32, tag="ivf")
            nc.scalar.activation(ivf[:], lstdf[:], AF.Exp, scale=-2.0)

            # ---------- beta transposes ----------
            psB = psmisc.tile([T, BL], f32, tag="pss", bufs=2)
            for i in range(4):
                nc.tensor.transpose(
                    psB[:, 128 * i : 128 * (i + 1)], betab[:, i, :], ident
                )
            betaT = sb.tile([T, BL], f32, tag="betaT")
            nc.vector.tensor_copy(betaT[:], psB[:])
            # bcat = [ln(1-beta); ln(beta)] on 64 partitions
            bcat = sb.tile([2 * T, BL], f32r, tag="bcat")
            nc.scalar.activation(bcat[0:T, :], betaT[:], AF.Ln, bias=1.0, scale=-1.0)
            nc.scalar.activation(bcat[T : 2 * T, :], betaT[:], AF.Ln)

            # ---------- x transposes + drains + interleaved main GEMM ------
            xT = {}
            xxT = {}
            pst_tiles = {}
            psG = psmisc.tile([T, BL], f32, tag="psg", bufs=1)

            def emit_transpose(j, h):
                pst = psx.tile([128, 256], f32, tag="pst", padded_shape=[128, 512])
                nc.tensor.transpose(
                    pst[:, 0:128], xb[2 * h][:, 128 * j : 128 * (j + 1)], ident
                )
                nc.tensor.transpose(
                    pst[:, 128:256], xb[2 * h + 1][:, 128 * j : 128 * (j + 1)], ident
                )
                pst_tiles[(j, h)] = pst

            def emit_drains(j, h):
                pst = pst_tiles[(j, h)]
                xt = xpool.tile([128, 256], f32r, tag=f"xT{j}_{h}")
                xx = xpool.tile([128, 256], f32r, tag=f"xxT{j}_{h}")
                # each drain reads PSUM once (two-PSUM-input ops are illegal,
                # and gpsimd cannot read PSUM at all)
                nc.vector.tensor_copy(xt[:], pst[:])
                nc.scalar.activation(xx[:], pst[:], AF.Square)
                xT[(j, h)] = xt
                xxT[(j, h)] = xx

            def emit_gemm(j, h):
                g = psG[:, 256 * h : 256 * (h + 1)]
                nc.tensor.matmul(
                    g, WT[:, 64 * j + 32 : 64 * j + 64], xxT[(j, h)][:],
                    start=(j == 0), stop=False,
                )
                nc.tensor.matmul(
                    g, WT[:, 64 * j : 64 * j + 32], xT[(j, h)][:],
                    start=False, stop=(j == NJ - 1),
                )

            WT = sb.tile([128, 512], f32r, tag="WT")

            order = [(j, h) for h in range(2) for j in range(NJ)]
            for (j, h) in order[:2]:
                emit_transpose(j, h)
                emit_drains(j, h)

            # W vector part, emitted after the first drains so the early
            # drains don't queue behind it on the vector engine.
            # w12f: cols 0:256 = w1 = -0.5*iv, cols 256:512 = w2 = mu*iv
            w12f = sb.tile([128, 512], f32, tag="w12f")
            nc.vector.tensor_scalar(w12f[:, 0:256], ivf[:], -0.5, None, ALU.mult)
            nc.vector.tensor_tensor(w12f[:, 256:512], muf[:], ivf[:], ALU.mult)
            wtmp = sb.tile([128, 256], f32, tag="wtmp")
            m2r = sb.tile([128, 1], f32, tag="m2r")
            nc.vector.scalar_tensor_tensor(
                wtmp[:], muf[:], 1.0, w12f[:, 256:512], ALU.mult, ALU.mult,
                accum_out=m2r[:],
            )

            for (j, h) in order[2:4]:
                emit_transpose(j, h)
                emit_drains(j, h)
            # cumsum matmul here: bcat is ready by now, TensorE doesn't stall
            psC = psmisc.tile([T, BL], f32, tag="pss", bufs=2)
            nc.tensor.matmul(psC[:], lcatr, bcat[:], start=True, stop=True)
            for (j, h) in order[4:6]:
                emit_transpose(j, h)
                emit_drains(j, h)

            # ---------- W unfold via on-chip transposes ----------
            # psWF chunk c' (128 cols) = transpose of w12f[:, 128c':128c'+128];
            # within a chunk, column index = (t*4 + s).
            psWF = psmisc.tile([128, 512], f32, tag="psw", bufs=1)
            for cpr in range(4):
                nc.tensor.transpose(
                    psWF[:, 128 * cpr : 128 * (cpr + 1)],
                    w12f[:, 128 * cpr : 128 * (cpr + 1)],
                    ident,
                )
            # k0 fold: psK = sum_s m2r[(t s)]  (plain f32: 1-col f32r mm is
            # rejected by the ISA checker)
            psK = psmisc.tile([T, 1], f32, tag="pss", bufs=2)
            nc.tensor.matmul(psK[:], sel4, m2r[:], start=True, stop=True)
            k0 = sb.tile([T, 1], f32, tag="k0")
            nc.vector.tensor_scalar(k0[:], psK[:], -0.5, float(D // 2), ALU.mult, ALU.add)

            # WT col layout per d-chunk j (= 2s + fh):
            #   [64j : 64j+32]    = w2T_j
            #   [64j+32 : 64j+64] = w1T_j
            wtv = WT.rearrange("p (s x) -> p s x", x=128)
            for kindbit in range(2):  # 0 = w2, 1 = w1
                for fh in range(2):
                    cpr = 2 * (1 - kindbit) + fh
                    src = psWF[:, 128 * cpr : 128 * (cpr + 1)].rearrange(
                        "p (t s) -> p s t", s=4
                    )
                    off = 64 * fh + 32 * kindbit
                    nc.vector.tensor_copy(wtv[:, :, off : off + 32], src)

            # rest of the transposes, GEMM chasing with lag 4
            LAG = 4
            gcur = 0
            for i, (j, h) in enumerate(order[6:], start=6):
                emit_transpose(j, h)
                emit_drains(j, h)
                while gcur <= i - LAG:
                    emit_gemm(*order[gcur])
                    gcur += 1
            while gcur < len(order):
                emit_gemm(*order[gcur])
                gcur += 1

            # ---------- lpi4 (stick-breaking, 128-partition stacked) ----------
            lpi4 = sb.tile([128, BL // 4], f32, tag="lpi4")
            npacc = sb.tile([T, 4], f32, tag="npacc")
            for blk in range(4):
                nc.vector.tensor_scalar(
                    lpi4[T * blk : T * (blk + 1), :],
                    psC[:, 128 * blk : 128 * (blk + 1)],
                    0.0, 0.0, ALU.add, ALU.add,
                    accum_out=npacc[:, blk : blk + 1],
                )

            # klg4 stacked [128, 128] w/ batch sums for N_g
            klg4 = sb.tile([128, BL // 4], f32, tag="klg4")
            ngacc = sb.tile([T, 4], f32, tag="ngacc")
            for blk in range(4):
                nc.vector.tensor_scalar(
                    klg4[T * blk : T * (blk + 1), :],
                    psG[:, 128 * blk : 128 * (blk + 1)],
                    k0[:], 0.0, ALU.add, ALU.add,
                    accum_out=ngacc[:, blk : blk + 1],
                )

            # ---------- assemble [T,2] partials, exchange across cores -----
            red4 = sb.tile([128, 2], f32, tag="red4")
            if use_remote:
                # partials land replicated 4x on the partitions, so every
                # peer's contribution arrives already in red4 layout
                nc.vector.reduce_sum(
                    ccs128[0:T, 0:1], ngacc[:], axis=mybir.AxisListType.X
                )
                nc.vector.reduce_sum(
                    ccs128[0:T, 1:2], npacc[:], axis=mybir.AxisListType.X
                )
                nc.vector.tensor_copy(ccs128[T : 2 * T, :], ccs128[0:T, :])
                nc.vector.tensor_copy(ccs128[2 * T : 4 * T, :], ccs128[0 : 2 * T, :])
                nc.vector.tensor_copy(gbuf[:, 0:2], ccs128[:])
                nc.gpsimd.trigger_dma(count=None)

                # diff4 overlaps the exchange
                diff4 = sb.tile([128, BL // 4], f32, tag="diff4")
                nc.vector.tensor_tensor(diff4[:], klg4[:], lpi4[:], ALU.subtract)

                # 7 remote writes x (16//8)=2 increments each
                nc.vector.wait_ge(rsem, 14)
                nc.vector.reduce_sum(
                    red4[:], gbuf.rearrange("p (d c) -> p c d", c=2),
                    axis=mybir.AxisListType.X,
                )
            else:
                ccs = sb.tile([T, 2], f32, tag="ccs")
                nc.vector.reduce_sum(ccs[:, 0:1], ngacc[:], axis=mybir.AxisListType.X)
                nc.vector.reduce_sum(ccs[:, 1:2], npacc[:], axis=mybir.AxisListType.X)

                ccin = dram.tile([T, 2], f32, tag="ccin")
                ccout = dram.tile([T, 2], f32, tag="ccout")
                nc.gpsimd.dma_start(ccin[:], ccs[:])
                if os.environ.get("DPVI_NO_CC"):
                    nc.gpsimd.dma_start(ccout[:], ccin[:])
                else:
                    nc.gpsimd.collective_compute(
                        "AllReduce",
                        ALU.add,
                        replica_groups=[list(range(NCORES))],
                        ins=[ccin.opt()],
                        outs=[ccout.opt()],
                    )

                # diff4 overlaps the collective
                diff4 = sb.tile([128, BL // 4], f32, tag="diff4")
                nc.vector.tensor_tensor(diff4[:], klg4[:], lpi4[:], ALU.subtract)

                # sync queue: idle at this point, so the readback DMA is not
                # queued behind the collective machinery on gpsimd
                nc.sync.dma_start(red4[0:T, :], ccout[:])
                nc.vector.tensor_copy(red4[T : 2 * T, :], red4[0:T, :])
                nc.vector.tensor_copy(red4[2 * T : 4 * T, :], red4[0 : 2 * T, :])

            rsum = sb.tile([128, 1], f32, tag="rsum")
            nc.vector.tensor_tensor(rsum[:], red4[:, 0:1], red4[:, 1:2], ALU.add)
            rinv = sb.tile([128, 1], f32, tag="rinv")
            nc.vector.reciprocal(rinv[:], rsum[:])
            mix4 = sb.tile([128, 1], f32, tag="mix4")
            nc.vector.tensor_tensor(mix4[:], red4[:, 1:2], rinv[:], ALU.mult)

            # kl = log_pi + mix*(kl_g - log_pi); phi-num = exp(kl)
            kl4 = sb.tile([128, BL // 4], f32, tag="kl4")
            nc.vector.scalar_tensor_tensor(
                kl4[:], diff4[:], mix4[:], lpi4[:], ALU.mult, ALU.add
            )
            e4 = sb.tile([128, BL // 4], f32r, tag="e4")
            nc.scalar.activation(e4[:], kl4[:], AF.Exp)
            mkl4 = sb.tile([128, BL // 4], f32r, tag="mkl4")
            nc.vector.scalar_tensor_tensor(
                mkl4[:], klg4[:], mix4[:], e4[:], ALU.mult, ALU.mult
            )

            # den / num per batch column (separate PSUM tiles: matmul out
            # base partition must be 0/32/64)
            psDe = psmisc.tile([4, BL // 4], f32, tag="pss", bufs=2)
            nc.tensor.matmul(psDe[:], selblkr, e4[:], start=True, stop=True)
            psDm = psmisc.tile([4, BL // 4], f32, tag="pss", bufs=2)
            nc.tensor.matmul(psDm[:], selblkr, mkl4[:], start=True, stop=True)
            nde = sb.tile([4, BL // 4], f32, tag="nde")
            nc.vector.tensor_copy(nde[:], psDe[:])
            rd = sb.tile([4, BL // 4], f32, tag="rd")
            nc.vector.reciprocal(rd[:], nde[:])
            ndn = sb.tile([4, BL // 4], f32, tag="ndn")
            nc.vector.tensor_copy(ndn[:], psDm[:])
            lik4 = sb.tile([4, BL // 4], f32, tag="lik4")
            liks = sb.tile([4, 1], f32, tag="liks")
            nc.vector.scalar_tensor_tensor(
                lik4[:], ndn[:], 1.0, rd[:], ALU.mult, ALU.mult,
                accum_out=liks[:],
            )
            psL = psmisc.tile([1, 1], f32, tag="pss", bufs=2)
            nc.tensor.matmul(psL[:], liks[:], ones4, start=True, stop=True)
            outsb = sb.tile([1, 1], f32, tag="outsb")
            nc.vector.tensor_copy(outsb[:], psL[:])
            nc.gpsimd.dma_start(out_d[:], outsb[:])

            if use_remote:
                # leave the manual semaphores clean for any later execution
                nc.gpsimd.sem_clear(rsem)
                nc.gpsimd.sem_clear(lsem)

    nc.compile()
    return nc


_NC_CACHE = None


def _get_nc():
    global _NC_CACHE
    if _NC_CACHE is None:
        _NC_CACHE = _build_nc()
    return _NC_CACHE


def _make_in_maps(x, mu, rho, beta_samples):
    x = np.ascontiguousarray(x, dtype=np.float32)
    mu = np.ascontiguousarray(mu, dtype=np.float32)
    rho = np.ascontiguousarray(rho, dtype=np.float32)
    beta = np.ascontiguousarray(beta_samples, dtype=np.float32)

    consts = np.zeros((128, CONSTW), dtype=np.float32)
    consts[:, 0:128] = np.eye(128, dtype=np.float32)
    consts[0:T, 128:160] = np.triu(np.ones((T, T), np.float32), 1)  # k<t
    consts[T : 2 * T, 128:160] = np.eye(T, dtype=np.float32)
    p = np.arange(128)
    consts[p, 160 + p // 4] = 1.0  # sel4
    consts[p, 192 + p // 32] = 1.0  # selblk / ones4

    in_maps = []
    for c in range(NCORES):
        in_maps.append(
            {
                "x": x[BL * c : BL * (c + 1)],
                "beta": beta[BL * c : BL * (c + 1)],
                "mu": mu,
                "rho": rho,
                "consts": consts,
            }
        )
    return in_maps


def run(inputs, trace=False, **kw):
    """Run on 8 NeuronCores; returns (result_scalar, BassKernelResults)."""
    from concourse.bass_utils import run_bass_kernel_spmd

    nc = _get_nc()
    in_maps = _make_in_maps(**inputs)
    res = run_bass_kernel_spmd(
        nc, in_maps, core_ids=list(range(NCORES)), trace=trace, **kw
    )
    total = 0.0
    for c in range(NCORES):
        total += float(res.results[c]["out"][0, 0])
    value = np.float32(total / B).reshape(())
    return value, res


def kernel(x, mu, rho, beta_samples):
    value, _ = run(dict(x=x, mu=mu, rho=rho, beta_samples=beta_samples))
    return value
